# revision 1
# baseline (speedup 1.0000x reference)
"""Trainium2 Bass kernel: modulated (StyleGAN2) 3x3 conv, groups=batch,
via Winograd F(2x2, 3x3).

Full-input contract: kernel(**inputs) takes the unsharded numpy inputs and
returns the full (16, 512, 64, 64) fp32 output. Batch sharded 2-per-core
across 8 NeuronCores; weights replicated.

Host prep (fp32 numpy, exact):
    s      = style @ mod_w.T + mod_b                  # (B, IC)
    xpl    = bf16(x * s), padded-column-PARITY-SPLIT  # (B, IC, 64, 2, 34)
    Wt     = bf16(G w G^T)                            # (IC, 16, OC) Winograd wts
    demod  = rsqrt(s^2 @ WS.T + eps*IC*K*K)           # (B, OC), SCALE folded

The parity split (padded col pc = 2k -> plane 0 slot k, pc = 2k+1 ->
plane 1 slot k, planes padded to 34 for 4B alignment) makes every DVE
access pattern unit-stride, enabling the 2-elem/cycle 16-bit mode and
avoiding SBUF fetch waste. Same trick on the output: the device writes a
planar (ty, r, parity, tx) bf16 layout; the host interleaves + upcasts.

Device per core (2 samples; PE does ONLY the 16-position batched matmuls):
    per 256-tile block: input transform B^T d B as two add/sub stages
    (stage A rows, stage B cols), 256 matmuls (16 pos x 4 oc x 4 ic chunk,
    N=256 bf16), ACT drains PSUM with the demod scale fused, vertical +
    horizontal output transform A^T m A as adds, contiguous DMA out.
"""

import sys

for _p in ("/opt/trn_rl_repo",):
    if _p not in sys.path:
        sys.path.append(_p)

import numpy as np
import ml_dtypes

import concourse.bass as bass
import concourse.tile as tile
from concourse import mybir
from concourse.bass_utils import run_bass_kernel_spmd

# ---------------------------------------------------------------------------
# Walrus workaround (see baseline): split >1 semaphore waits per instruction
# onto NoOp carriers.
# ---------------------------------------------------------------------------
import json as _json

_SPLIT_OK_ENGINES = {"PE", "DVE", "Activation", "Pool", "SP"}
_orig_to_json_bytes = bass.Bass.to_json_bytes


def _to_json_bytes_split_waits(self):
    raw = _orig_to_json_bytes(self)
    m = _json.loads(raw)
    changed = False
    for fn in m.get("functions", []):
        for bb in fn.get("blocks", []):
            insts = bb.get("instructions", [])
            new_insts = []
            for inst in insts:
                si = inst.get("sync_info")
                waits = (si or {}).get("on_wait") or []
                op = inst.get("opcode", "")
                limit = 2 if op == "EventSemaphore" else 1
                if len(waits) > limit:
                    eng = inst.get("engine")
                    assert eng in _SPLIT_OK_ENGINES, (
                        f"instruction {inst.get('name')} on engine {eng} has "
                        f"{len(waits)} waits; carrier NoOp not known-safe there"
                    )
                    changed = True
                    keep = waits[-limit:]
                    for i, w in enumerate(waits[:-limit]):
                        new_insts.append(
                            {
                                "debug": inst.get("debug", 0),
                                "engine": eng,
                                "ins": [],
                                "name": f"{inst['name']}.w{i}",
                                "opcode": "NoOp",
                                "outs": [],
                                "sync_info": {"on_wait": [w], "on_update": []},
                            }
                        )
                    si["on_wait"] = keep
                new_insts.append(inst)
            bb["instructions"] = new_insts
    if not changed:
        return raw
    return _json.dumps(m).encode()


bass.Bass.to_json_bytes = _to_json_bytes_split_waits

# ---------------------------------------------------------------------------
# Problem constants (hardcoded per spec)
# ---------------------------------------------------------------------------
B, IC, OC, H, W, KS, SD = 16, 512, 512, 64, 64, 3, 512
NCORES = 8
BPC = B // NCORES           # samples per core
P = 128
NIC = IC // P               # 4 ic chunks
NOC = OC // P               # 4 oc chunks
EPS_FOLDED = 1e-8 * IC * KS * KS

TYB = 8                     # tile-rows per block
TB = TYB * 32               # tiles per block = 256 (matmul free dim)
NBLK = (H // 2) // TYB      # 4 blocks per sample
BROWS = 2 * TYB + 2         # 18 padded rows per band
PL = 34                     # parity-plane width (33 used + 1 alignment pad)

F32 = mybir.dt.float32
BF16 = mybir.dt.bfloat16
ADD = mybir.AluOpType.add
SUB = mybir.AluOpType.subtract

BF = ml_dtypes.bfloat16

# Winograd transform matrices (host side)
G_MAT = np.array([[1, 0, 0], [0.5, 0.5, 0.5], [0.5, -0.5, 0.5], [0, 0, 1]], np.float32)


def build_nc():
    nc = bass.Bass()
    # x: scaled bf16, padded row+col parity planes:
    # [b, ic, row-parity, 33 row slots, col-parity(2) * 34 col slots]
    xpl = nc.dram_tensor("xpl", [BPC, IC, 2, 33, 2 * PL], BF16, kind="ExternalInput")
    xng = nc.dram_tensor("xng", [BPC, IC, 2, 33, 2 * PL], BF16, kind="ExternalInput")
    # host-precomputed block-0 input transforms (startup fast path)
    xt0d = nc.dram_tensor("xt0d", [P, 4, NIC, TB], BF16, kind="ExternalInput")
    va0d = nc.dram_tensor("va0d", [P, NIC, 4, TYB, 2 * PL], BF16, kind="ExternalInput")
    va1d = nc.dram_tensor("va1d", [P, NIC, 4, TYB, 2 * PL], BF16, kind="ExternalInput")
    # weights partition-major: [ki, hmaj(=h*4+u), c, oc] so each per-h DMA
    # moves 16KB-contiguous runs per partition
    wt = nc.dram_tensor("wt", [P, 16, NIC, OC], BF16, kind="ExternalInput")
    dT = nc.dram_tensor("dT", [OC, BPC], F32, kind="ExternalInput")
    # out: planar bf16 [b, r, oc, parity, ty, tx]; host interleaves+upcasts
    opl = nc.dram_tensor("opl", [BPC, 2, OC, 2, 32, 32], BF16, kind="ExternalOutput")


    with tile.TileContext(nc) as tc:
        with (
            tc.tile_pool(name="singles", bufs=1) as singles,
            tc.tile_pool(name="vap", bufs=2) as vap,
            tc.tile_pool(name="xtp", bufs=9) as xtp,
            tc.tile_pool(name="mp", bufs=3) as mp,
            tc.tile_pool(name="pp", bufs=2) as ppool,
            tc.tile_pool(name="ysp", bufs=2) as ysp,
            tc.tile_pool(name="tmpp", bufs=4) as tmpp,
            tc.tile_pool(name="psum", bufs=4, space="PSUM") as psum,
        ):
            # ---- constants (weight DMAs emitted in the prologue below so
            # the first band's DMAs aren't queued behind them) ---------------
            d_sb = singles.tile([P, NOC, BPC], F32)
            wt_sb = singles.tile([P, 16, NIC, OC], BF16)

            blocks = [(s, blk) for s in range(BPC) for blk in range(NBLK)]

            # ---- stage A via DMA: va[v] = rows_a(+x) then accum rows_b ----
            # (hw-DGE copy + sw-DGE accumulate-add; subtraction via the
            # host-negated copy xng). Reads x rows straight from DRAM.
            xpl_v = xpl.rearrange("b (c ki) rp r l -> b ki c rp r l", ki=P)
            xng_v = xng.rearrange("b (c ki) rp r l -> b ki c rp r l", ki=P)
            va_tiles = {}

            def stage_a(bi):
                s, blk = blocks[bi]
                j = TYB * blk
                va = vap.tile([P, NIC, 4, TYB, 2, PL], BF16, tag="va", name=f"va{bi}")
                va_tiles[bi] = va
                # (v, copy plane/rows, accum plane/rows): padded rows
                # r0=rp0[j..], r1=rp1[j..], r2=rp0[j+1..], r3=rp1[j+1..]
                plan = [
                    (0, xpl_v, 0, 0, xng_v, 0, 1),   # v0 = r0 - r2
                    (1, xpl_v, 1, 0, xpl_v, 0, 1),   # v1 = r1 + r2
                    (2, xpl_v, 0, 1, xng_v, 1, 0),   # v2 = r2 - r1
                    (3, xpl_v, 1, 0, xng_v, 1, 1),   # v3 = r1 - r3
                ]
                for v, srcc, rpc, offc, srca, rpa, offa in plan:
                    nc.sync.dma_start(
                        va[:, :, v],
                        srcc[s, :, :, rpc, j + offc : j + offc + TYB].rearrange(
                            "ki c r (q l) -> ki c r q l", l=PL
                        ),
                    )
                    nc.gpsimd.dma_start(
                        va[:, :, v],
                        srca[s, :, :, rpa, j + offa : j + offa + TYB].rearrange(
                            "ki c r (q l) -> ki c r q l", l=PL
                        ),
                        accum_op=ADD,
                    )

            # ---- stage B: horizontal input transform (all unit-stride) -----
            # padded col pc=2k -> plane0[k], pc=2k+1 -> plane1[k]
            #   c0 (pc=2tx)   = plane0[0:32]   c2 (pc=2tx+2) = plane0[1:33]
            #   c1 (pc=2tx+1) = plane1[0:32]   c3 (pc=2tx+3) = plane1[1:33]
            xt_tiles = {}

            def stage_b_one(bi, v, h):
                va = va_tiles[bi]
                xt = xtp.tile([P, NIC, TB], BF16, tag="xt", name=f"xt{bi}_{v}_{h}")
                xt_tiles[(bi, v, h)] = xt
                o = xt.rearrange("p c (ty tx) -> p c ty tx", tx=32)
                c0 = va[:, :, v, :, 0, 0:32]
                c1 = va[:, :, v, :, 1, 0:32]
                c2 = va[:, :, v, :, 0, 1:33]
                c3 = va[:, :, v, :, 1, 1:33]
                if h == 0:
                    nc.vector.tensor_tensor(o, c0, c2, SUB)
                elif h == 1:
                    nc.vector.tensor_tensor(o, c1, c2, ADD)
                elif h == 2:
                    nc.vector.tensor_tensor(o, c2, c1, SUB)
                else:
                    nc.vector.tensor_tensor(o, c1, c3, SUB)

            def stage_b(bi, h):
                for v in range(4):
                    stage_b_one(bi, v, h)

            # ---- deferred horizontal output transform + DMA out ------------
            pending = []

            def horizontal_flush(split=False):
                while pending:
                    pbi, pp, ys = pending.pop(0)
                    pb, pblk = blocks[pbi]
                    groups = [slice(0, 2), slice(2, 4)] if split else [slice(0, NOC)]
                    for g in groups:
                        for r in range(2):
                            pr = pp[:, g, r]            # [P, ng, 4, TB]
                            ye = ys[:, r, 0, g]         # [P, ng, TB] contiguous
                            yo = ys[:, r, 1, g]
                            ng = NOC if not split else 2
                            t3 = tmpp.tile([P, ng, TB], BF16, tag="t" if not split else "tq", name=f"t3_{pbi}_{r}_{g.start}")
                            nc.vector.tensor_tensor(t3, pr[:, :, 0, :], pr[:, :, 1, :], ADD)
                            nc.vector.tensor_tensor(ye, t3, pr[:, :, 2, :], ADD)
                            t4 = tmpp.tile([P, ng, TB], BF16, tag="t" if not split else "tq", name=f"t4_{pbi}_{r}_{g.start}")
                            nc.vector.tensor_tensor(t4, pr[:, :, 1, :], pr[:, :, 2, :], SUB)
                            nc.vector.tensor_tensor(yo, t4, pr[:, :, 3, :], SUB)
                        for o in range(NOC)[g]:
                            for r in range(2):
                                nc.sync.dma_start(
                                    opl[
                                        pb, r, o * P : (o + 1) * P, :,
                                        pblk * 8 : (pblk + 1) * 8,
                                    ],
                                    ys[:, r, :, o].rearrange(
                                        "p q (ty tx) -> p q ty tx", tx=32
                                    ),
                                )

            # ---- main loop -------------------------------------------------
            NB = len(blocks)
            # prologue DMA queue in strict first-use order: demod scales
            # (first drain), then block-0 h0 inputs interleaved with the h0
            # weight chunks, h1 weights, block-0 stage-A planes, the rest
            nc.sync.dma_start(d_sb, dT.rearrange("(o ki) b -> ki o b", ki=P))
            va0 = vap.tile([P, NIC, 4, TYB, 2, PL], BF16, tag="va", name="va0")
            va_tiles[0] = va0
            for v in range(4):
                xt = xtp.tile([P, NIC, TB], BF16, tag="xt", name=f"xt0_{v}_0")
                xt_tiles[(0, v, 0)] = xt
                nc.sync.dma_start(xt, xt0d[:, v])
                nc.sync.dma_start(wt_sb[:, v], wt[:, v])
            nc.sync.dma_start(
                va0, va0d.rearrange("ki c v r (q l) -> ki c v r q l", l=PL)
            )
            for hm in range(4, 8):
                nc.sync.dma_start(wt_sb[:, hm], wt[:, hm])
            va1 = vap.tile([P, NIC, 4, TYB, 2, PL], BF16, tag="va", name="va1")
            va_tiles[1] = va1
            nc.sync.dma_start(
                va1, va1d.rearrange("ki c v r (q l) -> ki c v r q l", l=PL)
            )
            for hm in range(8, 16):
                nc.sync.dma_start(wt_sb[:, hm], wt[:, hm])

            for bi in range(NB):
                s, blk = blocks[bi]
                b = s

                pp_t = ppool.tile([P, NOC, 2, 4, TB], BF16, tag="pp", name=f"pp{bi}")
                # [r, parity, o, ty*tx]
                ys_t = ysp.tile([P, 2, 2, NOC, TB], BF16, tag="ys", name=f"ys{bi}")

                for h in range(4):
                    # DVE pipeline prefetches (before this h's drains)
                    if h == 0 and bi + 2 < NB:
                        stage_a(bi + 2)
                    if h < 3:
                        stage_b(bi, h + 1)
                    elif bi + 1 < NB:
                        stage_b(bi + 1, 0)
                    if h == 1:
                        horizontal_flush()

                    m_t = mp.tile([P, NOC, 4, TB], BF16, tag="m", name=f"m{bi}_{h}")

                    for o in range(NOC):
                        for up in range(2):
                            ps = psum.tile([P, 2, 512], F32, tag="ps", name=f"ps{bi}_{h}_{o}_{up}")
                            for ui in range(2):
                                u = 2 * up + ui
                                xt = xt_tiles[(bi, u, h)]
                                for c in range(NIC):
                                    nc.tensor.matmul(
                                        ps[:, ui, :TB],
                                        wt_sb[:, 4 * h + u, c, o * P : (o + 1) * P],
                                        xt[:, c, :],
                                        start=(c == 0),
                                        stop=(c == NIC - 1),
                                    )
                            # drain both u-banks with demod scale (ACT)
                            nc.scalar.activation(
                                out=m_t[:, o, 2 * up : 2 * up + 2, :],
                                in_=ps[:, :, :TB],
                                func=mybir.ActivationFunctionType.Copy,
                                scale=d_sb[:, o, b : b + 1],
                            )

                    # vertical output transform: P0 = m0+m1+m2, P1 = m1-m2-m3
                    if bi < NB - 1 or h < 3:
                        t = tmpp.tile([P, NOC, TB], BF16, tag="t", name=f"tv{bi}_{h}")
                        nc.vector.tensor_tensor(t, m_t[:, :, 0, :], m_t[:, :, 1, :], ADD)
                        nc.vector.tensor_tensor(pp_t[:, :, 0, h, :], t, m_t[:, :, 2, :], ADD)
                        t2 = tmpp.tile([P, NOC, TB], BF16, tag="t", name=f"tv2{bi}_{h}")
                        nc.vector.tensor_tensor(t2, m_t[:, :, 2, :], m_t[:, :, 3, :], ADD)
                        nc.vector.tensor_tensor(pp_t[:, :, 1, h, :], m_t[:, :, 1, :], t2, SUB)
                    else:
                        # final h-block: per-oc-pair vertical, each followed
                        # immediately by that pair's horizontal + DMA out
                        for q in range(2):
                            sl = slice(2 * q, 2 * q + 2)
                            t = tmpp.tile([P, 2, TB], BF16, tag="tq", name=f"tvq{q}")
                            nc.vector.tensor_tensor(t, m_t[:, sl, 0, :], m_t[:, sl, 1, :], ADD)
                            nc.vector.tensor_tensor(pp_t[:, sl, 0, h, :], t, m_t[:, sl, 2, :], ADD)
                            t2 = tmpp.tile([P, 2, TB], BF16, tag="tq", name=f"tv2q{q}")
                            nc.vector.tensor_tensor(t2, m_t[:, sl, 2, :], m_t[:, sl, 3, :], ADD)
                            nc.vector.tensor_tensor(pp_t[:, sl, 1, h, :], m_t[:, sl, 1, :], t2, SUB)
                            for r in range(2):
                                pr = pp_t[:, sl, r]
                                ye = ys_t[:, r, 0, sl]
                                yo = ys_t[:, r, 1, sl]
                                t3 = tmpp.tile([P, 2, TB], BF16, tag="tq", name=f"ft3_{q}_{r}")
                                nc.vector.tensor_tensor(t3, pr[:, :, 0, :], pr[:, :, 1, :], ADD)
                                nc.vector.tensor_tensor(ye, t3, pr[:, :, 2, :], ADD)
                                t4 = tmpp.tile([P, 2, TB], BF16, tag="tq", name=f"ft4_{q}_{r}")
                                nc.vector.tensor_tensor(t4, pr[:, :, 1, :], pr[:, :, 2, :], SUB)
                                nc.vector.tensor_tensor(yo, t4, pr[:, :, 3, :], SUB)
                            for o in range(2 * q, 2 * q + 2):
                                for r in range(2):
                                    nc.sync.dma_start(
                                        opl[
                                            b, r, o * P : (o + 1) * P, :,
                                            blk * 8 : (blk + 1) * 8,
                                        ],
                                        ys_t[:, r, :, o].rearrange(
                                            "p q2 (ty tx) -> p q2 ty tx", tx=32
                                        ),
                                    )
                        last_done = True

                if bi < NB - 1:
                    pending.append((bi, pp_t, ys_t))

            horizontal_flush()

    return nc


_NC = None


def _get_nc():
    global _NC
    if _NC is None:
        _NC = build_nc()
    return _NC


def _host_prep(x, style, weight, mod_w, mod_b):
    x = np.asarray(x, np.float32)
    style = np.asarray(style, np.float32)
    w = np.asarray(weight, np.float32)[0]          # (OC, IC, 3, 3)
    mod_w = np.asarray(mod_w, np.float32)
    mod_b = np.asarray(mod_b, np.float32)

    s = style @ mod_w.T + mod_b                    # (B, IC)
    xs = (x * s[:, :, None, None]).astype(BF)      # (B, IC, H, W) bf16

    # padded row+col parity planes: padded row pr=2j -> row-plane0[j]
    # (= x row 2j-1), pr=2j+1 -> row-plane1[j] (= x row 2j); same for cols
    xpl = np.zeros((B, IC, 2, 33, 2, PL), dtype=BF)
    xpl[:, :, 0, 1:33, 0, 1:33] = xs[:, :, 1::2, 1::2]
    xpl[:, :, 0, 1:33, 1, 0:32] = xs[:, :, 1::2, 0::2]
    xpl[:, :, 1, 0:32, 0, 1:33] = xs[:, :, 0::2, 1::2]
    xpl[:, :, 1, 0:32, 1, 0:32] = xs[:, :, 0::2, 0::2]

    WS = (w * w).sum(axis=(2, 3))                  # (OC, IC)
    demod = 1.0 / np.sqrt((s * s) @ WS.T + EPS_FOLDED)   # (B, OC)

    Wt = np.einsum("uk,oikl,vl->oiuv", G_MAT, w, G_MAT)  # (OC, IC, 4, 4)
    # device layout [ki, hmaj=h*4+u, c, oc]: ic = c*128 + ki
    wt4 = Wt.reshape(OC, NIC, P, 4, 4)             # (oc, c, ki, u, h)
    wt = np.ascontiguousarray(
        wt4.transpose(2, 4, 3, 1, 0).reshape(P, 16, NIC, OC)
    ).astype(BF)                                   # (ki, h*4+u, c, oc)
    return xpl.reshape(B, IC, 2, 33, 2 * PL), wt, demod


def _blockN_va(xpl_core, blk):
    x = xpl_core.astype(np.float32).reshape(IC, 2, 33, 2, PL)
    j = TYB * blk
    r0 = x[:, 0, j : j + TYB]
    r1 = x[:, 1, j : j + TYB]
    r2 = x[:, 0, j + 1 : j + TYB + 1]
    r3 = x[:, 1, j + 1 : j + TYB + 1]
    va = np.stack([r0 - r2, r1 + r2, r2 - r1, r1 - r3], axis=1).astype(BF)
    return np.ascontiguousarray(
        va.reshape(NIC, P, 4, TYB, 2 * PL).transpose(1, 0, 2, 3, 4)
    )


def _block0_transforms(xpl_core):
    # xpl_core: (IC, 2, 33, 2*PL) bf16 for sample 0 of this core
    x = xpl_core.astype(np.float32).reshape(IC, 2, 33, 2, PL)
    r0 = x[:, 0, 0:TYB]          # (IC, 8, 2, PL)
    r1 = x[:, 1, 0:TYB]
    r2 = x[:, 0, 1 : TYB + 1]
    r3 = x[:, 1, 1 : TYB + 1]
    va = np.stack([r0 - r2, r1 + r2, r2 - r1, r1 - r3], axis=1).astype(BF)
    vaf = va.astype(np.float32)  # (IC, 4, 8, 2, PL)
    c0 = vaf[:, :, :, 0, 0:32]
    c1 = vaf[:, :, :, 1, 0:32]
    c2 = vaf[:, :, :, 0, 1:33]
    xt0 = (c0 - c2).astype(BF).reshape(IC, 4, TB)      # h=0, (ic, v, ty*tx)
    # device layouts: va0d [ki, c, v, ty, 2*PL]; xt0d [ki, v, c, TB]
    va0d = np.ascontiguousarray(
        va.reshape(NIC, P, 4, TYB, 2 * PL).transpose(1, 0, 2, 3, 4)
    )
    xt0d = np.ascontiguousarray(
        xt0.reshape(NIC, P, 4, TB).transpose(1, 2, 0, 3)
    )
    return va0d, xt0d


def make_in_maps(inputs):
    xpl, wt, demod = _host_prep(**inputs)
    xng = -xpl
    in_maps = []
    for i in range(NCORES):
        sl = slice(i * BPC, (i + 1) * BPC)
        va0d, xt0d = _block0_transforms(xpl[i * BPC])
        in_maps.append(
            {
                "xpl": np.ascontiguousarray(xpl[sl]),
                "xng": np.ascontiguousarray(xng[sl]),
                "wt": wt,
                "dT": np.ascontiguousarray(demod[sl].T),
                "va0d": va0d,
                "xt0d": xt0d,
                "va1d": _blockN_va(xpl[i * BPC], 1),
            }
        )
    return in_maps


def _post(res_list):
    # opl [BPC, r2, OC, p2, ty32, tx32] bf16 -> [BPC, OC, 64, 64] f32
    outs = []
    for r in res_list:
        a = np.asarray(r["opl"]).astype(np.float32)
        # -> [b, oc, ty, r, tx, p]
        a = a.transpose(0, 2, 4, 1, 5, 3).reshape(BPC, OC, H, W)
        outs.append(a)
    return np.concatenate(outs, axis=0)


def kernel(x, style, weight, mod_w, mod_b):
    in_maps = make_in_maps(
        dict(x=x, style=style, weight=weight, mod_w=mod_w, mod_b=mod_b)
    )
    nc = _get_nc()
    res = run_bass_kernel_spmd(nc, in_maps, core_ids=list(range(NCORES)))
    return _post(res.results)



# revision 2
# speedup vs baseline: 1.0179x; 1.0179x over previous
"""Trainium2 Bass kernel: modulated (StyleGAN2) 3x3 conv, groups=batch,
via Winograd F(4x4, 3x3) with fp16 matmuls.

Full-input contract: kernel(**inputs) takes the unsharded numpy inputs and
returns the full (16, 512, 64, 64) fp32 output. Batch sharded 2-per-core
across 8 NeuronCores; weights replicated.

Host prep (numpy):
    s   = style @ mod_w.T + mod_b                    # (B, IC)
    xs  = fp16(x * s)                                # fold modulation into x
    V   = fp16(B^T d B) over 6x6 tiles (stride 4)    # input transform
    U   = fp16(G w G^T / 16)                         # weight transform (shared
                                                     #  across samples + cores)
    demod applied on the host AFTER the device run (a positive
    per-(sample, out-channel) scale commutes with the conv + transforms).

Device per core (2 samples, N = 2*256 tiles = 512 moving columns):
    Stream V[pos], U[pos] (512 KB each) for the 36 Winograd positions,
    ordered v-major/u-minor with order [1,2,3,4,0,5] on both axes.
    Per position: 16 matmuls (4 oc chunks x 4 ic accumulation steps) into
    one 4-bank fp32 PSUM tile [128, 4oc, 512]; a single fat ACT drain
    converts it to fp16. The output transform A^T M A runs as fat fp16
    tensor ops ([128, 4oc, 512] stage 1 at u-group milestones;
    [128, 4p, 512] per oc stage 2 at v-group ends), so only y3 = T3 + z5
    (+ its DMA) trails the final matmul. Demod + scatter on the host.
"""

import sys

for _p in ("/opt/trn_rl_repo",):
    if _p not in sys.path:
        sys.path.append(_p)

import numpy as np

import concourse.bass as bass
import concourse.tile as tile
from concourse import mybir
from concourse.bass_utils import run_bass_kernel_spmd

# ---------------------------------------------------------------------------
# Walrus workaround (from baseline): split >1 semaphore waits per instruction
# onto NoOp carriers.
# ---------------------------------------------------------------------------
import json as _json

_SPLIT_OK_ENGINES = {"PE", "DVE", "Activation", "Pool", "SP"}
_orig_to_json_bytes = bass.Bass.to_json_bytes


def _to_json_bytes_split_waits(self):
    raw = _orig_to_json_bytes(self)
    m = _json.loads(raw)
    changed = False
    for fn in m.get("functions", []):
        for bb in fn.get("blocks", []):
            insts = bb.get("instructions", [])
            new_insts = []
            for inst in insts:
                si = inst.get("sync_info")
                waits = (si or {}).get("on_wait") or []
                op = inst.get("opcode", "")
                limit = 2 if op == "EventSemaphore" else 1
                if len(waits) > limit:
                    eng = inst.get("engine")
                    assert eng in _SPLIT_OK_ENGINES, (
                        f"instruction {inst.get('name')} on engine {eng} has "
                        f"{len(waits)} waits; carrier NoOp not known-safe there"
                    )
                    changed = True
                    keep = waits[-limit:]
                    for i, w in enumerate(waits[:-limit]):
                        new_insts.append(
                            {
                                "debug": inst.get("debug", 0),
                                "engine": eng,
                                "ins": [],
                                "name": f"{inst['name']}.w{i}",
                                "opcode": "NoOp",
                                "outs": [],
                                "sync_info": {"on_wait": [w], "on_update": []},
                            }
                        )
                    si["on_wait"] = keep
                new_insts.append(inst)
            bb["instructions"] = new_insts
    if not changed:
        return raw
    return _json.dumps(m).encode()


bass.Bass.to_json_bytes = _to_json_bytes_split_waits

# ---------------------------------------------------------------------------
# Problem constants (hardcoded per spec)
# ---------------------------------------------------------------------------
B, IC, OC, H, W, KS, SD = 16, 512, 512, 64, 64, 3, 512
NCORES = 8
BPC = B // NCORES           # samples per core
P = 128
NIC = IC // P               # 4 ic chunks
NOC = OC // P               # 4 oc chunks
EPS = 1e-8
USCALE = 1.0 / 16.0         # global weight scale, undone in host demod

M4, T6 = 4, 6               # F(4x4,3x3): output tile 4, input tile 6
NTY = H // M4               # 16 tile rows
NTX = W // M4               # 16 tile cols
NT = NTY * NTX              # 256 tiles per sample
NN = BPC * NT               # 512 moving columns per matmul

ORD = [1, 2, 3, 4, 0, 5]    # processing order for both u and v
POS = [(u, v) for v in ORD for u in ORD]
NPOS = len(POS)             # 36

F32 = mybir.dt.float32
FP16 = mybir.dt.float16
ADD = mybir.AluOpType.add
SUB = mybir.AluOpType.subtract
MUL = mybir.AluOpType.mult
COPY = mybir.ActivationFunctionType.Copy

F16 = np.float16

# Winograd F(4x4,3x3) matrices (Lavin points 0,1,-1,2,-2,inf)
BT_MAT = np.array(
    [
        [4, 0, -5, 0, 1, 0],
        [0, -4, -4, 1, 1, 0],
        [0, 4, -4, -1, 1, 0],
        [0, -2, -1, 2, 1, 0],
        [0, 2, -1, -2, 1, 0],
        [0, 4, 0, -5, 0, 1],
    ],
    np.float32,
)
G_MAT = np.array(
    [
        [1 / 4, 0, 0],
        [-1 / 6, -1 / 6, -1 / 6],
        [-1 / 6, 1 / 6, -1 / 6],
        [1 / 24, 1 / 12, 1 / 6],
        [1 / 24, -1 / 12, 1 / 6],
        [0, 0, 1],
    ],
    np.float32,
)
# A^T = [[1,1,1,1,1,0],[0,1,-1,2,-2,0],[0,1,1,4,4,0],[0,1,-1,8,-8,1]]
# implemented as the add/scale schedule below.


def build_nc():
    nc = bass.Bass()
    # position-ordered transformed input / weights: [pos, ki, chunk, *]
    vt = nc.dram_tensor("vt", [NPOS, P, NIC, NN], FP16, kind="ExternalInput")
    ut = nc.dram_tensor("ut", [NPOS, P, NIC, OC], FP16, kind="ExternalInput")
    # out: [occ, p, ki, q, n] fp16; host scatters + applies demod
    opl = nc.dram_tensor("opl", [NOC, M4, P, M4, NN], FP16, kind="ExternalOutput")

    with tile.TileContext(nc) as tc:
        with (
            tc.tile_pool(name="vp", bufs=4) as vp,
            tc.tile_pool(name="up", bufs=4) as up,
            tc.tile_pool(name="zp", bufs=2) as zp,
            tc.tile_pool(name="drp", bufs=3) as drp,
            tc.tile_pool(name="s1p", bufs=8) as s1p,
            tc.tile_pool(name="s2h", bufs=8) as s2h,
            tc.tile_pool(name="s2t", bufs=4) as s2t,
            tc.tile_pool(name="psum", bufs=2, space="PSUM") as psum,
        ):
            s2 = [dict() for _ in range(2)]     # per p-half stage-2 state

            def stage2(vv, z_t):
                """Fire stage-2 (contract v) fat contiguous ops for a completed
                v-group, one [P, 2, NOC, NN] op per p-half."""
                for h in range(2):
                    st = s2[h]
                    zs = z_t[:, 2 * h : 2 * h + 2]
                    if vv == 1:
                        st["z1"] = zs
                    elif vv == 2:
                        A = s2h.tile([P, 2, NOC, NN], FP16, tag="s2h", name=f"A{h}")
                        Bt = s2h.tile([P, 2, NOC, NN], FP16, tag="s2h", name=f"B{h}")
                        nc.vector.tensor_tensor(A, st["z1"], zs, ADD)
                        nc.vector.tensor_tensor(Bt, st["z1"], zs, SUB)
                        st["A"], st["B"] = A, Bt
                        del st["z1"]
                    elif vv == 3:
                        st["z3"] = zs
                    elif vv == 4:
                        C = s2h.tile([P, 2, NOC, NN], FP16, tag="s2h", name=f"C{h}")
                        E = s2t.tile([P, 2, NOC, NN], FP16, tag="s2t", name=f"E{h}")
                        nc.vector.tensor_tensor(C, st["z3"], zs, ADD)
                        nc.vector.tensor_tensor(E, st["z3"], zs, SUB)
                        del st["z3"]
                        y1 = s2t.tile([P, 2, NOC, NN], FP16, tag="s2t", name=f"y1{h}")
                        y2 = s2t.tile([P, 2, NOC, NN], FP16, tag="s2t", name=f"y2{h}")
                        T3 = s2h.tile([P, 2, NOC, NN], FP16, tag="s2h", name=f"T3{h}")
                        nc.vector.scalar_tensor_tensor(y1, E, 2.0, st["B"], MUL, ADD)
                        nc.vector.scalar_tensor_tensor(y2, C, 4.0, st["A"], MUL, ADD)
                        nc.vector.scalar_tensor_tensor(T3, E, 8.0, st["B"], MUL, ADD)
                        nc.sync.dma_start(
                            opl[:, 2 * h : 2 * h + 2, :, 1, :].rearrange(
                                "o p k n -> k p o n"
                            ),
                            y1,
                        )
                        nc.sync.dma_start(
                            opl[:, 2 * h : 2 * h + 2, :, 2, :].rearrange(
                                "o p k n -> k p o n"
                            ),
                            y2,
                        )
                        st["C"], st["T3"] = C, T3
                        del st["B"]
                    elif vv == 0:
                        T = s2t.tile([P, 2, NOC, NN], FP16, tag="s2t", name=f"T{h}")
                        y0 = s2t.tile([P, 2, NOC, NN], FP16, tag="s2t", name=f"y0{h}")
                        nc.vector.tensor_tensor(T, zs, st["A"], ADD)
                        nc.vector.tensor_tensor(y0, T, st["C"], ADD)
                        nc.sync.dma_start(
                            opl[:, 2 * h : 2 * h + 2, :, 0, :].rearrange(
                                "o p k n -> k p o n"
                            ),
                            y0,
                        )
                        del st["A"], st["C"]
                    elif vv == 5:
                        y3 = s2t.tile([P, 2, NOC, NN], FP16, tag="s2t", name=f"y3{h}")
                        nc.vector.tensor_tensor(y3, st["T3"], zs, ADD)
                        nc.sync.dma_start(
                            opl[:, 2 * h : 2 * h + 2, :, 3, :].rearrange(
                                "o p k n -> k p o n"
                            ),
                            y3,
                        )
                        del st["T3"]

            # ---- main loop over positions --------------------------------
            s1 = {}
            z_t = None
            for i, (u, v) in enumerate(POS):
                v_sb = vp.tile([P, NIC, NN], FP16, tag="v", name=f"v{i}")
                u_sb = up.tile([P, NIC, OC], FP16, tag="u", name=f"u{i}")
                nc.sync.dma_start(v_sb, vt[i])
                nc.sync.dma_start(u_sb, ut[i])

                if u == ORD[0]:
                    s1 = {}
                    z_t = zp.tile([P, M4, NOC, NN], FP16, tag="z", name=f"z{v}")

                ps = psum.tile([P, NOC, NN], F32, tag="ps", name=f"ps{i}")
                for o in range(NOC):
                    for c in range(NIC):
                        nc.tensor.matmul(
                            ps[:, o, :],
                            u_sb[:, c, o * P : (o + 1) * P],
                            v_sb[:, c, :],
                            start=(c == 0),
                            stop=(c == NIC - 1),
                        )
                md = drp.tile([P, NOC, NN], FP16, tag="dr", name=f"md{i}")
                nc.scalar.activation(out=md, in_=ps, func=COPY)

                if u == 1:
                    s1["m1"] = md
                elif u == 2:
                    a = s1p.tile([P, NOC, NN], FP16, tag="s1", name=f"a{v}")
                    b = s1p.tile([P, NOC, NN], FP16, tag="s1", name=f"b{v}")
                    nc.vector.tensor_tensor(a, s1["m1"], md, ADD)
                    nc.vector.tensor_tensor(b, s1["m1"], md, SUB)
                    s1["a"], s1["b"] = a, b
                    del s1["m1"]
                elif u == 3:
                    s1["m3"] = md
                elif u == 4:
                    cc = s1p.tile([P, NOC, NN], FP16, tag="s1", name=f"c{v}")
                    e = s1p.tile([P, NOC, NN], FP16, tag="s1", name=f"e{v}")
                    nc.vector.tensor_tensor(cc, s1["m3"], md, ADD)
                    nc.vector.tensor_tensor(e, s1["m3"], md, SUB)
                    del s1["m3"]
                    t3 = s1p.tile([P, NOC, NN], FP16, tag="s1", name=f"t3{v}")
                    nc.vector.scalar_tensor_tensor(
                        z_t[:, 1], e, 2.0, s1["b"], MUL, ADD
                    )
                    nc.vector.scalar_tensor_tensor(
                        z_t[:, 2], cc, 4.0, s1["a"], MUL, ADD
                    )
                    nc.vector.scalar_tensor_tensor(t3, e, 8.0, s1["b"], MUL, ADD)
                    s1["c"], s1["t3"] = cc, t3
                    del s1["b"]
                elif u == 0:
                    t = s1p.tile([P, NOC, NN], FP16, tag="s1", name=f"t{v}")
                    nc.vector.tensor_tensor(t, md, s1["a"], ADD)
                    nc.vector.tensor_tensor(z_t[:, 0], t, s1["c"], ADD)
                    del s1["a"], s1["c"]
                elif u == 5:
                    nc.vector.tensor_tensor(z_t[:, 3], s1["t3"], md, ADD)
                    del s1["t3"]

                if u == ORD[-1]:
                    stage2(v, z_t)

    return nc


_NC = None


def _get_nc():
    global _NC
    if _NC is None:
        _NC = build_nc()
    return _NC


def _host_prep(x, style, weight, mod_w, mod_b):
    x = np.asarray(x, np.float32)
    style = np.asarray(style, np.float32)
    w = np.asarray(weight, np.float32)[0]          # (OC, IC, 3, 3)
    mod_w = np.asarray(mod_w, np.float32)
    mod_b = np.asarray(mod_b, np.float32)

    s = style @ mod_w.T + mod_b                    # (B, IC)
    xs = (x * s[:, :, None, None]).astype(F16).astype(np.float32)

    # demod (SCALE cancels; fold USCALE undo here)
    WS = (w * w).sum(axis=(2, 3))                  # (OC, IC)
    demod = (1.0 / np.sqrt((s * s) @ WS.T + EPS * IC * KS * KS)) / USCALE

    # input transform: 6x6 tiles, stride 4, pad 1
    xp = np.zeros((B, IC, H + 2, W + 2), np.float32)
    xp[:, :, 1:-1, 1:-1] = xs
    idx = np.arange(NTY)[:, None] * M4 + np.arange(T6)[None, :]
    tiles = xp[:, :, idx[:, None, :, None], idx[None, :, None, :]]
    # (B, IC, ty, tx, 6u, 6v) -- two-stage to halve the einsum cost
    v1 = np.einsum("uk,bcijkl->bciujl", BT_MAT, tiles)
    V = np.einsum("vl,bciujl->bcijuv", BT_MAT, v1).astype(F16)

    # weight transform
    U = (np.einsum("uk,oikl,vl->uvio", G_MAT, w, G_MAT) * USCALE).astype(F16)
    U5 = U.reshape(T6, T6, NIC, P, OC)             # (u, v, cc, ki, oc)
    ut = np.empty((NPOS, P, NIC, OC), dtype=F16)
    for i, (u, v) in enumerate(POS):
        ut[i] = U5[u, v].transpose(1, 0, 2)        # (ki, cc, oc)

    return V, ut, demod


_LAST_DEMOD = None


def make_in_maps(inputs):
    global _LAST_DEMOD
    V, ut, demod = _host_prep(**inputs)
    _LAST_DEMOD = demod
    # V: (B, IC, ty, tx, u, v) fp16
    in_maps = []
    for core in range(NCORES):
        sl = slice(core * BPC, (core + 1) * BPC)
        Vc = V[sl]                                 # (2, 512, 16, 16, 6, 6)
        Vr = Vc.reshape(BPC, NIC, P, NT, T6, T6)   # (s, cc, ki, ij, u, v)
        vt = np.empty((NPOS, P, NIC, NN), dtype=F16)
        for i, (u, v) in enumerate(POS):
            # (s, cc, ki, ij) -> (ki, cc, s*ij)
            vt[i] = (
                Vr[:, :, :, :, u, v]
                .transpose(2, 1, 0, 3)
                .reshape(P, NIC, NN)
            )
        in_maps.append({"vt": np.ascontiguousarray(vt), "ut": ut})
    return in_maps


def _post(res_list, demod=None):
    if demod is None:
        demod = _LAST_DEMOD
    outs = []
    for core, r in enumerate(res_list):
        a = np.asarray(r["opl"]).astype(np.float32)
        # [occ, p, ki, q, n] -> (s, occ, ki, ty, p, tx, q)
        a = a.reshape(NOC, M4, P, M4, BPC, NTY, NTX)
        a = a.transpose(4, 0, 2, 5, 1, 6, 3).reshape(BPC, OC, H, W)
        sl = slice(core * BPC, (core + 1) * BPC)
        a *= demod[sl][:, :, None, None]
        outs.append(a)
    return np.concatenate(outs, axis=0)


def kernel(x, style, weight, mod_w, mod_b):
    in_maps = make_in_maps(
        dict(x=x, style=style, weight=weight, mod_w=mod_w, mod_b=mod_b)
    )
    nc = _get_nc()
    res = run_bass_kernel_spmd(nc, in_maps, core_ids=list(range(NCORES)))
    return _post(res.results)


# revision 3
# speedup vs baseline: 1.0279x; 1.0098x over previous
"""Trainium2 Bass kernel: modulated (StyleGAN2) 3x3 conv, groups=batch,
via Winograd F(4x4, 3x3) with fp16 matmuls.

Full-input contract: kernel(**inputs) takes the unsharded numpy inputs and
returns the full (16, 512, 64, 64) fp32 output. Batch sharded 2-per-core
across 8 NeuronCores; weights replicated.

Host prep (numpy):
    s   = style @ mod_w.T + mod_b                    # (B, IC)
    xs  = fp16(x * s)                                # fold modulation into x
    V   = fp16(B^T d B) over 6x6 tiles (stride 4)    # input transform
    U   = fp16(G w G^T / 16)                         # weight transform (shared
                                                     #  across samples + cores)
    demod applied on the host AFTER the device run (a positive
    per-(sample, out-channel) scale commutes with the conv + transforms).

Device per core (2 samples, N = 2*256 tiles = 512 moving columns):
    Stream V[pos], U[pos] (512 KB each) for the 36 Winograd positions,
    ordered v-major/u-minor with order [1,2,3,4,0,5] on both axes.
    Per position: 16 matmuls (4 oc chunks x 4 ic accumulation steps) into
    one 4-bank fp32 PSUM tile [128, 4oc, 512]; a single fat ACT drain
    converts it to fp16. The output transform A^T M A runs as fat fp16
    tensor ops ([128, 4oc, 512] stage 1 at u-group milestones;
    [128, 4p, 512] per oc stage 2 at v-group ends), so only y3 = T3 + z5
    (+ its DMA) trails the final matmul. Demod + scatter on the host.
"""

import sys

for _p in ("/opt/trn_rl_repo",):
    if _p not in sys.path:
        sys.path.append(_p)

import numpy as np

import concourse.bass as bass
import concourse.tile as tile
from concourse import mybir
from concourse.bass_utils import run_bass_kernel_spmd

# ---------------------------------------------------------------------------
# Walrus workaround (from baseline): split >1 semaphore waits per instruction
# onto NoOp carriers.
# ---------------------------------------------------------------------------
import json as _json

_SPLIT_OK_ENGINES = {"PE", "DVE", "Activation", "Pool", "SP"}
_orig_to_json_bytes = bass.Bass.to_json_bytes


def _to_json_bytes_split_waits(self):
    raw = _orig_to_json_bytes(self)
    m = _json.loads(raw)
    changed = False
    for fn in m.get("functions", []):
        for bb in fn.get("blocks", []):
            insts = bb.get("instructions", [])
            new_insts = []
            for inst in insts:
                si = inst.get("sync_info")
                waits = (si or {}).get("on_wait") or []
                op = inst.get("opcode", "")
                limit = 2 if op == "EventSemaphore" else 1
                if len(waits) > limit:
                    eng = inst.get("engine")
                    assert eng in _SPLIT_OK_ENGINES, (
                        f"instruction {inst.get('name')} on engine {eng} has "
                        f"{len(waits)} waits; carrier NoOp not known-safe there"
                    )
                    changed = True
                    keep = waits[-limit:]
                    for i, w in enumerate(waits[:-limit]):
                        new_insts.append(
                            {
                                "debug": inst.get("debug", 0),
                                "engine": eng,
                                "ins": [],
                                "name": f"{inst['name']}.w{i}",
                                "opcode": "NoOp",
                                "outs": [],
                                "sync_info": {"on_wait": [w], "on_update": []},
                            }
                        )
                    si["on_wait"] = keep
                new_insts.append(inst)
            bb["instructions"] = new_insts
    if not changed:
        return raw
    return _json.dumps(m).encode()


bass.Bass.to_json_bytes = _to_json_bytes_split_waits

# ---------------------------------------------------------------------------
# Problem constants (hardcoded per spec)
# ---------------------------------------------------------------------------
B, IC, OC, H, W, KS, SD = 16, 512, 512, 64, 64, 3, 512
NCORES = 8
BPC = B // NCORES           # samples per core
P = 128
NIC = IC // P               # 4 ic chunks
NOC = OC // P               # 4 oc chunks
EPS = 1e-8
USCALE = 1.0 / 16.0         # global weight scale, undone in host demod

M4, T6 = 4, 6               # F(4x4,3x3): output tile 4, input tile 6
NTY = H // M4               # 16 tile rows
NTX = W // M4               # 16 tile cols
NT = NTY * NTX              # 256 tiles per sample
NN = BPC * NT               # 512 moving columns per matmul

ORD = [1, 2, 3, 4, 0, 5]    # processing order for both u and v
POS = [(u, v) for v in ORD for u in ORD]
NPOS = len(POS)             # 36

F32 = mybir.dt.float32
FP16 = mybir.dt.float16
ADD = mybir.AluOpType.add
SUB = mybir.AluOpType.subtract
MUL = mybir.AluOpType.mult
COPY = mybir.ActivationFunctionType.Copy

F16 = np.float16

# Winograd F(4x4,3x3) matrices (Lavin points 0,1,-1,2,-2,inf)
BT_MAT = np.array(
    [
        [4, 0, -5, 0, 1, 0],
        [0, -4, -4, 1, 1, 0],
        [0, 4, -4, -1, 1, 0],
        [0, -2, -1, 2, 1, 0],
        [0, 2, -1, -2, 1, 0],
        [0, 4, 0, -5, 0, 1],
    ],
    np.float32,
)
G_MAT = np.array(
    [
        [1 / 4, 0, 0],
        [-1 / 6, -1 / 6, -1 / 6],
        [-1 / 6, 1 / 6, -1 / 6],
        [1 / 24, 1 / 12, 1 / 6],
        [1 / 24, -1 / 12, 1 / 6],
        [0, 0, 1],
    ],
    np.float32,
)
# A^T = [[1,1,1,1,1,0],[0,1,-1,2,-2,0],[0,1,1,4,4,0],[0,1,-1,8,-8,1]]
# implemented as the add/scale schedule below.


def build_nc():
    nc = bass.Bass()
    # position-ordered transformed input / weights: [pos, ki, chunk, *]
    vt = nc.dram_tensor("vt", [NPOS, P, NIC, NN], FP16, kind="ExternalInput")
    ut = nc.dram_tensor("ut", [NPOS, P, NIC, OC], FP16, kind="ExternalInput")
    # out: [occ, p, ki, q, n] fp16; host scatters + applies demod
    opl = nc.dram_tensor("opl", [NOC, M4, P, M4, NN], FP16, kind="ExternalOutput")

    with tile.TileContext(nc) as tc:
        with (
            tc.tile_pool(name="vp", bufs=3) as vp,
            tc.tile_pool(name="up", bufs=2) as up,
            tc.tile_pool(name="zp", bufs=2) as zp,
            tc.tile_pool(name="drp", bufs=4) as drp,
            tc.tile_pool(name="s1p", bufs=8) as s1p,
            tc.tile_pool(name="s1a", bufs=2) as s1a,
            tc.tile_pool(name="s2h", bufs=8) as s2h,
            tc.tile_pool(name="s2t", bufs=4) as s2t,
            tc.tile_pool(name="psum", bufs=2, space="PSUM") as psum,
        ):
            s2 = [dict() for _ in range(2)]     # per p-half stage-2 state

            def stage2(vv, z_t):
                """Fire stage-2 (contract v) fat contiguous ops for a completed
                v-group, one [P, 2, NOC, NN] op per p-half."""
                for h in range(2):
                    st = s2[h]
                    zs = z_t[:, 2 * h : 2 * h + 2]
                    if vv == 1:
                        st["z1"] = zs
                    elif vv == 2:
                        A = s2h.tile([P, 2, NOC, NN], FP16, tag="s2h", name=f"A{h}")
                        Bt = s2h.tile([P, 2, NOC, NN], FP16, tag="s2h", name=f"B{h}")
                        nc.vector.tensor_tensor(A, st["z1"], zs, ADD)
                        nc.vector.tensor_tensor(Bt, st["z1"], zs, SUB)
                        st["A"], st["B"] = A, Bt
                        del st["z1"]
                    elif vv == 3:
                        st["z3"] = zs
                    elif vv == 4:
                        C = s2h.tile([P, 2, NOC, NN], FP16, tag="s2h", name=f"C{h}")
                        E = s2t.tile([P, 2, NOC, NN], FP16, tag="s2t", name=f"E{h}")
                        nc.vector.tensor_tensor(C, st["z3"], zs, ADD)
                        nc.vector.tensor_tensor(E, st["z3"], zs, SUB)
                        del st["z3"]
                        y1 = s2t.tile([P, 2, NOC, NN], FP16, tag="s2t", name=f"y1{h}")
                        y2 = s2t.tile([P, 2, NOC, NN], FP16, tag="s2t", name=f"y2{h}")
                        T3 = s2h.tile([P, 2, NOC, NN], FP16, tag="s2h", name=f"T3{h}")
                        E2 = s2t.tile([P, 2, NOC, NN], FP16, tag="s2t", name=f"E2{h}")
                        E8 = s2t.tile([P, 2, NOC, NN], FP16, tag="s2t", name=f"E8{h}")
                        C4 = s2t.tile([P, 2, NOC, NN], FP16, tag="s2t", name=f"C4{h}")
                        nc.vector.tensor_scalar_mul(E2, E, 2.0)
                        nc.vector.tensor_scalar_mul(E8, E, 8.0)
                        nc.vector.tensor_scalar_mul(C4, C, 4.0)
                        nc.vector.tensor_tensor(y1, st["B"], E2, ADD)
                        nc.vector.tensor_tensor(y2, st["A"], C4, ADD)
                        nc.vector.tensor_tensor(T3, st["B"], E8, ADD)
                        nc.sync.dma_start(
                            opl[:, 2 * h : 2 * h + 2, :, 1, :].rearrange(
                                "o p k n -> k p o n"
                            ),
                            y1,
                        )
                        nc.sync.dma_start(
                            opl[:, 2 * h : 2 * h + 2, :, 2, :].rearrange(
                                "o p k n -> k p o n"
                            ),
                            y2,
                        )
                        st["C"], st["T3"] = C, T3
                        del st["B"]
                    elif vv == 0:
                        T = s2t.tile([P, 2, NOC, NN], FP16, tag="s2t", name=f"T{h}")
                        y0 = s2t.tile([P, 2, NOC, NN], FP16, tag="s2t", name=f"y0{h}")
                        nc.vector.tensor_tensor(T, zs, st["A"], ADD)
                        nc.vector.tensor_tensor(y0, T, st["C"], ADD)
                        nc.sync.dma_start(
                            opl[:, 2 * h : 2 * h + 2, :, 0, :].rearrange(
                                "o p k n -> k p o n"
                            ),
                            y0,
                        )
                        del st["A"], st["C"]
                    elif vv == 5:
                        y3 = s2t.tile([P, 2, NOC, NN], FP16, tag="s2t", name=f"y3{h}")
                        nc.vector.tensor_tensor(y3, st["T3"], zs, ADD)
                        nc.sync.dma_start(
                            opl[:, 2 * h : 2 * h + 2, :, 3, :].rearrange(
                                "o p k n -> k p o n"
                            ),
                            y3,
                        )
                        del st["T3"]

            # ---- main loop over positions --------------------------------
            s1 = {}
            z_t = None
            for i, (u, v) in enumerate(POS):
                v_sb = vp.tile([P, NIC, NN], FP16, tag="v", name=f"v{i}")
                u_sb = up.tile([P, NIC, OC], FP16, tag="u", name=f"u{i}")
                nc.sync.dma_start(v_sb, vt[i])
                nc.sync.dma_start(u_sb, ut[i])

                if u == ORD[0]:
                    s1 = {}
                    z_t = zp.tile([P, M4, NOC, NN], FP16, tag="z", name=f"z{v}")

                ps = psum.tile([P, NOC, NN], F32, tag="ps", name=f"ps{i}")
                for o in range(NOC):
                    for c in range(NIC):
                        nc.tensor.matmul(
                            ps[:, o, :],
                            u_sb[:, c, o * P : (o + 1) * P],
                            v_sb[:, c, :],
                            start=(c == 0),
                            stop=(c == NIC - 1),
                        )
                md = drp.tile([P, NOC, NN], FP16, tag="dr", name=f"md{i}")
                nc.scalar.activation(out=md, in_=ps, func=COPY)

                if u == 1:
                    s1["m1"] = md
                elif u == 2:
                    a = s1a.tile([P, NOC, NN], FP16, tag="s1a", name=f"a{v}")
                    b = s1p.tile([P, NOC, NN], FP16, tag="s1", name=f"b{v}")
                    nc.vector.tensor_tensor(a, s1["m1"], md, ADD)
                    nc.vector.tensor_tensor(b, s1["m1"], md, SUB)
                    s1["a"], s1["b"] = a, b
                    del s1["m1"]
                elif u == 3:
                    s1["m3"] = md
                elif u == 4:
                    cc = s1p.tile([P, NOC, NN], FP16, tag="s1", name=f"c{v}")
                    e = s1p.tile([P, NOC, NN], FP16, tag="s1", name=f"e{v}")
                    nc.vector.tensor_tensor(cc, s1["m3"], md, ADD)
                    nc.vector.tensor_tensor(e, s1["m3"], md, SUB)
                    del s1["m3"]
                    t3 = s1p.tile([P, NOC, NN], FP16, tag="s1", name=f"t3{v}")
                    e2 = s1p.tile([P, NOC, NN], FP16, tag="s1", name=f"e2{v}")
                    e8 = s1p.tile([P, NOC, NN], FP16, tag="s1", name=f"e8{v}")
                    c4 = s1p.tile([P, NOC, NN], FP16, tag="s1", name=f"c4{v}")
                    nc.vector.tensor_scalar_mul(e2, e, 2.0)
                    nc.vector.tensor_scalar_mul(e8, e, 8.0)
                    nc.vector.tensor_scalar_mul(c4, cc, 4.0)
                    nc.vector.tensor_tensor(z_t[:, 1], s1["b"], e2, ADD)
                    nc.vector.tensor_tensor(z_t[:, 2], s1["a"], c4, ADD)
                    nc.vector.tensor_tensor(t3, s1["b"], e8, ADD)
                    s1["c"], s1["t3"] = cc, t3
                    del s1["b"]
                elif u == 0:
                    t = s1p.tile([P, NOC, NN], FP16, tag="s1", name=f"t{v}")
                    nc.vector.tensor_tensor(t, md, s1["a"], ADD)
                    nc.vector.tensor_tensor(z_t[:, 0], t, s1["c"], ADD)
                    del s1["a"], s1["c"]
                elif u == 5:
                    nc.vector.tensor_tensor(z_t[:, 3], s1["t3"], md, ADD)
                    del s1["t3"]

                if u == ORD[-1]:
                    stage2(v, z_t)

    return nc


_NC = None


def _get_nc():
    global _NC
    if _NC is None:
        _NC = build_nc()
    return _NC


def _host_prep(x, style, weight, mod_w, mod_b):
    x = np.asarray(x, np.float32)
    style = np.asarray(style, np.float32)
    w = np.asarray(weight, np.float32)[0]          # (OC, IC, 3, 3)
    mod_w = np.asarray(mod_w, np.float32)
    mod_b = np.asarray(mod_b, np.float32)

    s = style @ mod_w.T + mod_b                    # (B, IC)
    xs = (x * s[:, :, None, None]).astype(F16).astype(np.float32)

    # demod (SCALE cancels; fold USCALE undo here)
    WS = (w * w).sum(axis=(2, 3))                  # (OC, IC)
    demod = (1.0 / np.sqrt((s * s) @ WS.T + EPS * IC * KS * KS)) / USCALE

    # input transform: 6x6 tiles, stride 4, pad 1
    xp = np.zeros((B, IC, H + 2, W + 2), np.float32)
    xp[:, :, 1:-1, 1:-1] = xs
    idx = np.arange(NTY)[:, None] * M4 + np.arange(T6)[None, :]
    tiles = xp[:, :, idx[:, None, :, None], idx[None, :, None, :]]
    # (B, IC, ty, tx, 6u, 6v) -- two-stage to halve the einsum cost
    v1 = np.einsum("uk,bcijkl->bciujl", BT_MAT, tiles)
    V = np.einsum("vl,bciujl->bcijuv", BT_MAT, v1).astype(F16)

    # weight transform
    U = (np.einsum("uk,oikl,vl->uvio", G_MAT, w, G_MAT) * USCALE).astype(F16)
    U5 = U.reshape(T6, T6, NIC, P, OC)             # (u, v, cc, ki, oc)
    ut = np.empty((NPOS, P, NIC, OC), dtype=F16)
    for i, (u, v) in enumerate(POS):
        ut[i] = U5[u, v].transpose(1, 0, 2)        # (ki, cc, oc)

    return V, ut, demod


_LAST_DEMOD = None


def make_in_maps(inputs):
    global _LAST_DEMOD
    V, ut, demod = _host_prep(**inputs)
    _LAST_DEMOD = demod
    # V: (B, IC, ty, tx, u, v) fp16
    in_maps = []
    for core in range(NCORES):
        sl = slice(core * BPC, (core + 1) * BPC)
        Vc = V[sl]                                 # (2, 512, 16, 16, 6, 6)
        Vr = Vc.reshape(BPC, NIC, P, NT, T6, T6)   # (s, cc, ki, ij, u, v)
        vt = np.empty((NPOS, P, NIC, NN), dtype=F16)
        for i, (u, v) in enumerate(POS):
            # (s, cc, ki, ij) -> (ki, cc, s*ij)
            vt[i] = (
                Vr[:, :, :, :, u, v]
                .transpose(2, 1, 0, 3)
                .reshape(P, NIC, NN)
            )
        in_maps.append({"vt": np.ascontiguousarray(vt), "ut": ut})
    return in_maps


def _post(res_list, demod=None):
    if demod is None:
        demod = _LAST_DEMOD
    outs = []
    for core, r in enumerate(res_list):
        a = np.asarray(r["opl"]).astype(np.float32)
        # [occ, p, ki, q, n] -> (s, occ, ki, ty, p, tx, q)
        a = a.reshape(NOC, M4, P, M4, BPC, NTY, NTX)
        a = a.transpose(4, 0, 2, 5, 1, 6, 3).reshape(BPC, OC, H, W)
        sl = slice(core * BPC, (core + 1) * BPC)
        a *= demod[sl][:, :, None, None]
        outs.append(a)
    return np.concatenate(outs, axis=0)


def kernel(x, style, weight, mod_w, mod_b):
    in_maps = make_in_maps(
        dict(x=x, style=style, weight=weight, mod_w=mod_w, mod_b=mod_b)
    )
    nc = _get_nc()
    res = run_bass_kernel_spmd(nc, in_maps, core_ids=list(range(NCORES)))
    return _post(res.results)


# revision 4
# speedup vs baseline: 1.0481x; 1.0196x over previous
"""Trainium2 Bass kernel: modulated (StyleGAN2) 3x3 conv, groups=batch,
via Winograd F(4x4, 3x3) with fp16 matmuls.

Full-input contract: kernel(**inputs) takes the unsharded numpy inputs and
returns the full (16, 512, 64, 64) fp32 output. Batch sharded 2-per-core
across 8 NeuronCores; weights replicated.

Host prep (numpy):
    s   = style @ mod_w.T + mod_b                    # (B, IC)
    xs  = fp16(x * s)                                # fold modulation into x
    V   = fp16(B^T d B) over 6x6 tiles (stride 4)    # input transform
    U   = fp16(G w G^T / 16)                         # weight transform (shared
                                                     #  across samples + cores)
    demod applied on the host AFTER the device run (a positive
    per-(sample, out-channel) scale commutes with the conv + transforms).

Device per core (2 samples, N = 2*256 tiles = 512 moving columns):
    Stream V[pos], U[pos] (512 KB each) for the 36 Winograd positions,
    ordered v-major/u-minor with order [1,2,3,4,0,5] on both axes.
    Per position: 16 matmuls (4 oc chunks x 4 ic accumulation steps) into
    one 4-bank fp32 PSUM tile [128, 4oc, 512]; a single fat ACT drain
    converts it to fp16. The output transform A^T M A runs as fat fp16
    tensor ops ([128, 4oc, 512] stage 1 at u-group milestones;
    [128, 4p, 512] per oc stage 2 at v-group ends), so only y3 = T3 + z5
    (+ its DMA) trails the final matmul. Demod + scatter on the host.
"""

import sys

for _p in ("/opt/trn_rl_repo",):
    if _p not in sys.path:
        sys.path.append(_p)

import numpy as np

import concourse.bass as bass
import concourse.tile as tile
from concourse import mybir
from concourse.bass_utils import run_bass_kernel_spmd

# ---------------------------------------------------------------------------
# Walrus workaround (from baseline): split >1 semaphore waits per instruction
# onto NoOp carriers.
# ---------------------------------------------------------------------------
import json as _json

_SPLIT_OK_ENGINES = {"PE", "DVE", "Activation", "Pool", "SP"}
_orig_to_json_bytes = bass.Bass.to_json_bytes


def _to_json_bytes_split_waits(self):
    raw = _orig_to_json_bytes(self)
    m = _json.loads(raw)
    changed = False
    for fn in m.get("functions", []):
        for bb in fn.get("blocks", []):
            insts = bb.get("instructions", [])
            new_insts = []
            for inst in insts:
                si = inst.get("sync_info")
                waits = (si or {}).get("on_wait") or []
                op = inst.get("opcode", "")
                limit = 2 if op == "EventSemaphore" else 1
                if len(waits) > limit:
                    eng = inst.get("engine")
                    assert eng in _SPLIT_OK_ENGINES, (
                        f"instruction {inst.get('name')} on engine {eng} has "
                        f"{len(waits)} waits; carrier NoOp not known-safe there"
                    )
                    changed = True
                    keep = waits[-limit:]
                    for i, w in enumerate(waits[:-limit]):
                        new_insts.append(
                            {
                                "debug": inst.get("debug", 0),
                                "engine": eng,
                                "ins": [],
                                "name": f"{inst['name']}.w{i}",
                                "opcode": "NoOp",
                                "outs": [],
                                "sync_info": {"on_wait": [w], "on_update": []},
                            }
                        )
                    si["on_wait"] = keep
                new_insts.append(inst)
            bb["instructions"] = new_insts
    if not changed:
        return raw
    return _json.dumps(m).encode()


bass.Bass.to_json_bytes = _to_json_bytes_split_waits

# ---------------------------------------------------------------------------
# Problem constants (hardcoded per spec)
# ---------------------------------------------------------------------------
B, IC, OC, H, W, KS, SD = 16, 512, 512, 64, 64, 3, 512
NCORES = 8
BPC = B // NCORES           # samples per core
P = 128
NIC = IC // P               # 4 ic chunks
NOC = OC // P               # 4 oc chunks
EPS = 1e-8
USCALE = 1.0 / 16.0         # global weight scale, undone in host demod

M4, T6 = 4, 6               # F(4x4,3x3): output tile 4, input tile 6
NTY = H // M4               # 16 tile rows
NTX = W // M4               # 16 tile cols
NT = NTY * NTX              # 256 tiles per sample
NN = BPC * NT               # 512 moving columns per matmul

ORD = [1, 2, 3, 4, 0, 5]    # processing order for both u and v
POS = [(u, v) for v in ORD for u in ORD]
NPOS = len(POS)             # 36

F32 = mybir.dt.float32
FP16 = mybir.dt.float16
ADD = mybir.AluOpType.add
SUB = mybir.AluOpType.subtract
MUL = mybir.AluOpType.mult
COPY = mybir.ActivationFunctionType.Copy

F16 = np.float16

# Winograd F(4x4,3x3) matrices (Lavin points 0,1,-1,2,-2,inf)
BT_MAT = np.array(
    [
        [4, 0, -5, 0, 1, 0],
        [0, -4, -4, 1, 1, 0],
        [0, 4, -4, -1, 1, 0],
        [0, -2, -1, 2, 1, 0],
        [0, 2, -1, -2, 1, 0],
        [0, 4, 0, -5, 0, 1],
    ],
    np.float32,
)
G_MAT = np.array(
    [
        [1 / 4, 0, 0],
        [-1 / 6, -1 / 6, -1 / 6],
        [-1 / 6, 1 / 6, -1 / 6],
        [1 / 24, 1 / 12, 1 / 6],
        [1 / 24, -1 / 12, 1 / 6],
        [0, 0, 1],
    ],
    np.float32,
)
# A^T = [[1,1,1,1,1,0],[0,1,-1,2,-2,0],[0,1,1,4,4,0],[0,1,-1,8,-8,1]]
# implemented as the add/scale schedule below.


def build_nc():
    nc = bass.Bass()
    # position-ordered transformed input / weights: [pos, ki, chunk, *]
    vt = nc.dram_tensor("vt", [NPOS, P, NIC, NN], FP16, kind="ExternalInput")
    ut = nc.dram_tensor("ut", [NPOS, P, NIC, OC], FP16, kind="ExternalInput")
    # out: [occ, p, ki, q, n] fp16; host scatters + applies demod
    opl = nc.dram_tensor("opl", [NOC, M4, P, M4, NN], FP16, kind="ExternalOutput")

    with tile.TileContext(nc) as tc:
        with (
            tc.tile_pool(name="vp", bufs=3) as vp,
            tc.tile_pool(name="up", bufs=3) as up,
            tc.tile_pool(name="zp", bufs=2) as zp,
            tc.tile_pool(name="drp", bufs=4) as drp,
            tc.tile_pool(name="s1p", bufs=7) as s1p,
            tc.tile_pool(name="s1a", bufs=2) as s1a,
            tc.tile_pool(name="s2h", bufs=8) as s2h,
            tc.tile_pool(name="s2t", bufs=4) as s2t,
            tc.tile_pool(name="psum", bufs=2, space="PSUM") as psum,
        ):
            s2 = [dict() for _ in range(2)]     # per p-half stage-2 state

            def stage2(vv, z_t):
                """Fire stage-2 (contract v) fat contiguous ops for a completed
                v-group, one [P, 2, NOC, NN] op per p-half."""
                for h in range(2):
                    st = s2[h]
                    zs = z_t[:, 2 * h : 2 * h + 2]
                    if vv == 1:
                        st["z1"] = zs
                    elif vv == 2:
                        A = s2h.tile([P, 2, NOC, NN], FP16, tag="s2h", name=f"A{h}")
                        Bt = s2h.tile([P, 2, NOC, NN], FP16, tag="s2h", name=f"B{h}")
                        nc.vector.tensor_tensor(A, st["z1"], zs, ADD)
                        nc.vector.tensor_tensor(Bt, st["z1"], zs, SUB)
                        st["A"], st["B"] = A, Bt
                        del st["z1"]
                    elif vv == 3:
                        st["z3"] = zs
                    elif vv == 4:
                        C = s2h.tile([P, 2, NOC, NN], FP16, tag="s2h", name=f"C{h}")
                        E = s2t.tile([P, 2, NOC, NN], FP16, tag="s2t", name=f"E{h}")
                        nc.vector.tensor_tensor(C, st["z3"], zs, ADD)
                        nc.vector.tensor_tensor(E, st["z3"], zs, SUB)
                        del st["z3"]
                        y1 = s2t.tile([P, 2, NOC, NN], FP16, tag="s2t", name=f"y1{h}")
                        y2 = s2t.tile([P, 2, NOC, NN], FP16, tag="s2t", name=f"y2{h}")
                        T3 = s2h.tile([P, 2, NOC, NN], FP16, tag="s2h", name=f"T3{h}")
                        E2 = s2t.tile([P, 2, NOC, NN], FP16, tag="s2t", name=f"E2{h}")
                        E8 = s2t.tile([P, 2, NOC, NN], FP16, tag="s2t", name=f"E8{h}")
                        C4 = s2t.tile([P, 2, NOC, NN], FP16, tag="s2t", name=f"C4{h}")
                        nc.vector.tensor_scalar_mul(E2, E, 2.0)
                        nc.vector.tensor_scalar_mul(E8, E, 8.0)
                        nc.vector.tensor_scalar_mul(C4, C, 4.0)
                        nc.vector.tensor_tensor(y1, st["B"], E2, ADD)
                        nc.vector.tensor_tensor(y2, st["A"], C4, ADD)
                        nc.vector.tensor_tensor(T3, st["B"], E8, ADD)
                        nc.sync.dma_start(
                            opl[:, 2 * h : 2 * h + 2, :, 1, :].rearrange(
                                "o p k n -> k p o n"
                            ),
                            y1,
                        )
                        nc.sync.dma_start(
                            opl[:, 2 * h : 2 * h + 2, :, 2, :].rearrange(
                                "o p k n -> k p o n"
                            ),
                            y2,
                        )
                        st["C"], st["T3"] = C, T3
                        del st["B"]
                    elif vv == 0:
                        T = s2t.tile([P, 2, NOC, NN], FP16, tag="s2t", name=f"T{h}")
                        y0 = s2t.tile([P, 2, NOC, NN], FP16, tag="s2t", name=f"y0{h}")
                        nc.vector.tensor_tensor(T, zs, st["A"], ADD)
                        nc.vector.tensor_tensor(y0, T, st["C"], ADD)
                        nc.sync.dma_start(
                            opl[:, 2 * h : 2 * h + 2, :, 0, :].rearrange(
                                "o p k n -> k p o n"
                            ),
                            y0,
                        )
                        del st["A"], st["C"]
                    elif vv == 5:
                        y3 = s2t.tile([P, 2, NOC, NN], FP16, tag="s2t", name=f"y3{h}")
                        nc.vector.tensor_tensor(y3, st["T3"], zs, ADD)
                        nc.sync.dma_start(
                            opl[:, 2 * h : 2 * h + 2, :, 3, :].rearrange(
                                "o p k n -> k p o n"
                            ),
                            y3,
                        )
                        del st["T3"]

            # ---- main loop over positions --------------------------------
            s1 = {}
            z_t = None
            for i, (u, v) in enumerate(POS):
                v_sb = vp.tile([P, NIC, NN], FP16, tag="v", name=f"v{i}")
                u_sb = up.tile([P, NIC, OC], FP16, tag="u", name=f"u{i}")
                nc.sync.dma_start(v_sb, vt[i])
                nc.sync.dma_start(u_sb, ut[i])

                if u == ORD[0]:
                    s1 = {}
                    z_t = zp.tile([P, M4, NOC, NN], FP16, tag="z", name=f"z{v}")

                ps = psum.tile([P, NOC, NN], F32, tag="ps", name=f"ps{i}")
                for o in range(NOC):
                    for c in range(NIC):
                        nc.tensor.matmul(
                            ps[:, o, :],
                            u_sb[:, c, o * P : (o + 1) * P],
                            v_sb[:, c, :],
                            start=(c == 0),
                            stop=(c == NIC - 1),
                        )
                md = drp.tile([P, NOC, NN], FP16, tag="dr", name=f"md{i}")
                nc.scalar.activation(out=md, in_=ps, func=COPY)

                if u == 1:
                    s1["m1"] = md
                elif u == 2:
                    a = s1a.tile([P, NOC, NN], FP16, tag="s1a", name=f"a{v}")
                    b = s1p.tile([P, NOC, NN], FP16, tag="s1", name=f"b{v}")
                    nc.vector.tensor_tensor(a, s1["m1"], md, ADD)
                    nc.vector.tensor_tensor(b, s1["m1"], md, SUB)
                    s1["a"], s1["b"] = a, b
                    del s1["m1"]
                elif u == 3:
                    s1["m3"] = md
                elif u == 4:
                    cc = s1p.tile([P, NOC, NN], FP16, tag="s1", name=f"c{v}")
                    e = s1p.tile([P, NOC, NN], FP16, tag="s1", name=f"e{v}")
                    nc.vector.tensor_tensor(cc, s1["m3"], md, ADD)
                    nc.vector.tensor_tensor(e, s1["m3"], md, SUB)
                    del s1["m3"]
                    t3 = s1p.tile([P, NOC, NN], FP16, tag="s1", name=f"t3{v}")
                    e2 = s1p.tile([P, NOC, NN], FP16, tag="s1", name=f"e2{v}")
                    e8 = s1p.tile([P, NOC, NN], FP16, tag="s1", name=f"e8{v}")
                    c4 = s1p.tile([P, NOC, NN], FP16, tag="s1", name=f"c4{v}")
                    nc.vector.tensor_scalar_mul(e2, e, 2.0)
                    nc.vector.tensor_scalar_mul(e8, e, 8.0)
                    nc.vector.tensor_scalar_mul(c4, cc, 4.0)
                    nc.vector.tensor_tensor(z_t[:, 1], s1["b"], e2, ADD)
                    nc.vector.tensor_tensor(z_t[:, 2], s1["a"], c4, ADD)
                    nc.vector.tensor_tensor(t3, s1["b"], e8, ADD)
                    s1["c"], s1["t3"] = cc, t3
                    del s1["b"]
                elif u == 0:
                    t = s1p.tile([P, NOC, NN], FP16, tag="s1", name=f"t{v}")
                    nc.vector.tensor_tensor(t, md, s1["a"], ADD)
                    nc.vector.tensor_tensor(z_t[:, 0], t, s1["c"], ADD)
                    del s1["a"], s1["c"]
                elif u == 5:
                    nc.vector.tensor_tensor(z_t[:, 3], s1["t3"], md, ADD)
                    del s1["t3"]

                if u == ORD[-1]:
                    stage2(v, z_t)

    return nc


_NC = None


def _get_nc():
    global _NC
    if _NC is None:
        _NC = build_nc()
    return _NC


def _host_prep(x, style, weight, mod_w, mod_b):
    x = np.asarray(x, np.float32)
    style = np.asarray(style, np.float32)
    w = np.asarray(weight, np.float32)[0]          # (OC, IC, 3, 3)
    mod_w = np.asarray(mod_w, np.float32)
    mod_b = np.asarray(mod_b, np.float32)

    s = style @ mod_w.T + mod_b                    # (B, IC)
    xs = (x * s[:, :, None, None]).astype(F16).astype(np.float32)

    # demod (SCALE cancels; fold USCALE undo here)
    WS = (w * w).sum(axis=(2, 3))                  # (OC, IC)
    demod = (1.0 / np.sqrt((s * s) @ WS.T + EPS * IC * KS * KS)) / USCALE

    # input transform: 6x6 tiles, stride 4, pad 1
    xp = np.zeros((B, IC, H + 2, W + 2), np.float32)
    xp[:, :, 1:-1, 1:-1] = xs
    idx = np.arange(NTY)[:, None] * M4 + np.arange(T6)[None, :]
    tiles = xp[:, :, idx[:, None, :, None], idx[None, :, None, :]]
    # (B, IC, ty, tx, 6u, 6v) -- two-stage to halve the einsum cost
    v1 = np.einsum("uk,bcijkl->bciujl", BT_MAT, tiles)
    V = np.einsum("vl,bciujl->bcijuv", BT_MAT, v1).astype(F16)

    # weight transform
    U = (np.einsum("uk,oikl,vl->uvio", G_MAT, w, G_MAT) * USCALE).astype(F16)
    U5 = U.reshape(T6, T6, NIC, P, OC)             # (u, v, cc, ki, oc)
    ut = np.empty((NPOS, P, NIC, OC), dtype=F16)
    for i, (u, v) in enumerate(POS):
        ut[i] = U5[u, v].transpose(1, 0, 2)        # (ki, cc, oc)

    return V, ut, demod


_LAST_DEMOD = None


def make_in_maps(inputs):
    global _LAST_DEMOD
    V, ut, demod = _host_prep(**inputs)
    _LAST_DEMOD = demod
    # V: (B, IC, ty, tx, u, v) fp16
    in_maps = []
    for core in range(NCORES):
        sl = slice(core * BPC, (core + 1) * BPC)
        Vc = V[sl]                                 # (2, 512, 16, 16, 6, 6)
        Vr = Vc.reshape(BPC, NIC, P, NT, T6, T6)   # (s, cc, ki, ij, u, v)
        vt = np.empty((NPOS, P, NIC, NN), dtype=F16)
        for i, (u, v) in enumerate(POS):
            # (s, cc, ki, ij) -> (ki, cc, s*ij)
            vt[i] = (
                Vr[:, :, :, :, u, v]
                .transpose(2, 1, 0, 3)
                .reshape(P, NIC, NN)
            )
        in_maps.append({"vt": np.ascontiguousarray(vt), "ut": ut})
    return in_maps


def _post(res_list, demod=None):
    if demod is None:
        demod = _LAST_DEMOD
    outs = []
    for core, r in enumerate(res_list):
        a = np.asarray(r["opl"]).astype(np.float32)
        # [occ, p, ki, q, n] -> (s, occ, ki, ty, p, tx, q)
        a = a.reshape(NOC, M4, P, M4, BPC, NTY, NTX)
        a = a.transpose(4, 0, 2, 5, 1, 6, 3).reshape(BPC, OC, H, W)
        sl = slice(core * BPC, (core + 1) * BPC)
        a *= demod[sl][:, :, None, None]
        outs.append(a)
    return np.concatenate(outs, axis=0)


def kernel(x, style, weight, mod_w, mod_b):
    in_maps = make_in_maps(
        dict(x=x, style=style, weight=weight, mod_w=mod_w, mod_b=mod_b)
    )
    nc = _get_nc()
    res = run_bass_kernel_spmd(nc, in_maps, core_ids=list(range(NCORES)))
    return _post(res.results)
